# revision 1
# baseline (speedup 1.0000x reference)
"""Decoder block (rmsnorm->MHA(rope on Q,V)->W_O residual->rmsnorm->MLP residual)
on 8 Trainium2 NeuronCores.

Sharding: each core computes attention for 2 of the 16 heads over BOTH batches
(weights sharded by head), then one 8-rank AllToAll redistributes head outputs
so each core owns one (batch, 512-token-block) slice for the W_O projection,
second rmsnorm and MLP (full weights, token-sharded). Host concatenates the 8
token-block outputs.

Precision: fp16 storage/matmuls everywhere (fp32 PSUM accumulate, fp32
residual/stats). Measured end-to-end relative error vs the fp32 reference is
~3e-4.
"""

import os

import numpy as np

B, S, D, H = 2, 2048, 2048, 16
DH = 128
NC = 8
HPC = 2  # heads per core
P = 128
TB = 512  # token block (= S/4) and q-chunk width
KC = D // P  # 16 contraction chunks over D
FC = (4 * D) // P  # 64 contraction chunks over the MLP hidden dim
EPS = 1e-8
THETA = 10000.0

_CACHE = {}


def _install_ntff_hook():
    """Optional: register the axon NTFF profiling hook so trace=True works."""
    import sys
    import types

    if "antenv.axon_hooks" in sys.modules:
        return True
    try:
        mod = types.ModuleType("antenv.axon_hooks")
        _hook = [None]
        mod.set_axon_ntff_profile_hook = lambda h: _hook.__setitem__(0, h)
        mod.get_axon_ntff_profile_hook = lambda: _hook[0]
        import antenv
        from trn_agent_boot.trn_boot import _ntff_profile_via_ctypes

        sys.modules["antenv.axon_hooks"] = mod
        antenv.axon_hooks = mod
        mod.set_axon_ntff_profile_hook(
            _ntff_profile_via_ctypes("/opt/axon/libaxon_pjrt.so")
        )
        return True
    except Exception:
        return False


def _build():
    import concourse.bass as bass
    import concourse.mybir as mybir
    import concourse.tile as tile
    from concourse import bacc
    from concourse.masks import make_identity
    from contextlib import ExitStack

    f32 = mybir.dt.float32
    f16 = mybir.dt.float16
    AF = mybir.ActivationFunctionType
    OP = mybir.AluOpType

    nc = bacc.Bacc("TRN2", target_bir_lowering=False, debug=False, num_devices=NC)

    x_nat = nc.dram_tensor("x_nat", [B, S, D], f32, kind="ExternalInput")
    x_res = nc.dram_tensor("x_res", [TB, D], f32, kind="ExternalInput")
    wq = nc.dram_tensor("wq", [D, HPC * P], f16, kind="ExternalInput")
    wk = nc.dram_tensor("wk", [D, HPC * P], f16, kind="ExternalInput")
    wv = nc.dram_tensor("wv", [D, HPC * P], f16, kind="ExternalInput")
    wo = nc.dram_tensor("wo", [D, D], f16, kind="ExternalInput")
    w1 = nc.dram_tensor("w1", [D, 4 * D], f16, kind="ExternalInput")
    w2 = nc.dram_tensor("w2", [4 * D, D], f16, kind="ExternalInput")
    b1s = nc.dram_tensor("b1s", [P, FC], f32, kind="ExternalInput")
    b2 = nc.dram_tensor("b2", [1, D], f32, kind="ExternalInput")
    cos_qt = nc.dram_tensor("cos_qt", [64, S], f16, kind="ExternalInput")
    sin_qt = nc.dram_tensor("sin_qt", [64, S], f16, kind="ExternalInput")
    cos_v = nc.dram_tensor("cos_v", [S, 64], f16, kind="ExternalInput")
    sin_v = nc.dram_tensor("sin_v", [S, 64], f16, kind="ExternalInput")
    masks = nc.dram_tensor("masks", [4, P, TB], f16, kind="ExternalInput")
    out_d = nc.dram_tensor("out", [TB, D], f32, kind="ExternalOutput")

    inv_sqrt_dh = float(1.0 / np.sqrt(DH))

    with tile.TileContext(nc) as tc, ExitStack() as ctx:
        cst = ctx.enter_context(tc.tile_pool(name="cst", bufs=1))
        dram = ctx.enter_context(tc.tile_pool(name="dram", bufs=1, space="DRAM"))
        # long-lived across phases 3-4
        h2Tp = ctx.enter_context(tc.tile_pool(name="h2Tp", bufs=1))

        eps_t = cst.tile([P, 1], f32)
        nc.vector.memset(eps_t, EPS)
        ident16 = cst.tile([P, P], f16)
        make_identity(nc, ident16)
        ones_c = cst.tile([P, 1], f16)
        nc.vector.memset(ones_c, 1.0)
        b1_sb = cst.tile([P, FC], f32)
        nc.sync.dma_start(b1_sb, b1s.ap())
        with tc.tile_pool(name="wrm", bufs=1, space="PSUM") as wrmp:
            wrm = wrmp.tile([P, P], f32)
            for _ in range(24):
                nc.tensor.matmul(wrm, ident16, ident16, start=True, stop=True)

        # internal DRAM
        h_d = []
        for b in range(B):
            for qc in range(4):
                h_d.append(dram.tile([TB, D], f16, name=f"h_d{b}_{qc}"))
        a2a_in = [
            dram.tile([NC, P, TB], f16, name=f"a2a_in{h}") for h in range(HPC)
        ]
        a2a_out = [
            dram.tile([NC, P, TB], f16, name=f"a2a_out{h}") for h in range(HPC)
        ]
        x2_d = dram.tile([TB, D], f32, name="x2_d")

        # ---------- phase 1+2: rmsnorm1 fused with QKV/attention ----------
        with ExitStack() as p2:
            xst = p2.enter_context(tc.tile_pool(name="xst", bufs=3))
            scrp = p2.enter_context(tc.tile_pool(name="scrp", bufs=1))
            smp = p2.enter_context(tc.tile_pool(name="smp", bufs=4))
            hnat = p2.enter_context(tc.tile_pool(name="hnat", bufs=2))
            acst = p2.enter_context(tc.tile_pool(name="acst", bufs=1))
            hTp = p2.enter_context(tc.tile_pool(name="hTp", bufs=2))
            qrk = p2.enter_context(tc.tile_pool(name="qrk", bufs=1))
            vsb = p2.enter_context(tc.tile_pool(name="vsb", bufs=1))
            rtmp = p2.enter_context(tc.tile_pool(name="rtmp", bufs=2))
            vtmp = p2.enter_context(tc.tile_pool(name="vtmp", bufs=2))
            exps = p2.enter_context(tc.tile_pool(name="exps", bufs=6))
            rdp = p2.enter_context(tc.tile_pool(name="rdp", bufs=2))
            rdBp = p2.enter_context(tc.tile_pool(name="rdBp", bufs=2))
            stg = p2.enter_context(tc.tile_pool(name="stg", bufs=4))
            qkps = p2.enter_context(tc.tile_pool(name="qkps", bufs=2, space="PSUM"))
            vps = p2.enter_context(tc.tile_pool(name="vps", bufs=1, space="PSUM"))
            scps = p2.enter_context(tc.tile_pool(name="scps", bufs=2, space="PSUM"))
            avps = p2.enter_context(tc.tile_pool(name="avps", bufs=2, space="PSUM"))
            dnps = p2.enter_context(tc.tile_pool(name="dnps", bufs=1, space="PSUM"))
            wq_sb = acst.tile([P, KC, HPC * P], f16)
            nc.sync.dma_start(wq_sb, wq.rearrange("(c p) m -> p c m", p=P))
            wk_sb = acst.tile([P, KC, HPC * P], f16)
            nc.sync.dma_start(wk_sb, wk.rearrange("(c p) m -> p c m", p=P))
            wv_sb = acst.tile([P, KC, HPC * P], f16)
            nc.sync.dma_start(wv_sb, wv.rearrange("(c p) m -> p c m", p=P))
            cosq = acst.tile([64, S], f16)
            nc.sync.dma_start(cosq, cos_qt.ap())
            sinq = acst.tile([64, S], f16)
            nc.sync.dma_start(sinq, sin_qt.ap())
            cosv = acst.tile([P, KC, 64], f16)
            nc.sync.dma_start(cosv, cos_v.rearrange("(i p) f -> p i f", p=P))
            sinv = acst.tile([P, KC, 64], f16)
            nc.sync.dma_start(sinv, sin_v.rearrange("(i p) f -> p i f", p=P))
            maskt = acst.tile([P, 4, TB], f16)
            nc.sync.dma_start(maskt, masks.rearrange("m p t -> p m t"))

            for b in range(B):
                QR = {}
                KK = {}
                for h in range(HPC):
                    QR[h] = qrk.tile([P, S], f16, tag=f"qr{h}", name=f"qr{b}_{h}")
                    KK[h] = qrk.tile([P, S], f16, tag=f"kk{h}", name=f"kk{b}_{h}")
                VV = vsb.tile([P, KC, HPC, P], f16, tag="v", name=f"vv{b}")

                for qc in range(4):
                    for i in range(4 * qc, 4 * qc + 4):
                        xt_ = xst.tile([P, D], f32, tag="x", name=f"x{b}_{i}")
                        nc.sync.dma_start(xt_, x_nat.ap()[b, i * P : (i + 1) * P, :])
                        s_ = scrp.tile([P, D], f32, tag="s", name=f"s{b}_{i}")
                        ssq = smp.tile([P, 1], f32, tag="ssq", name=f"ssq{b}_{i}")
                        nc.scalar.activation(s_, xt_, AF.Square, accum_out=ssq)
                        rms_ = smp.tile([P, 1], f32, tag="rms", name=f"rms{b}_{i}")
                        nc.scalar.activation(
                            rms_, ssq, AF.Sqrt, bias=eps_t, scale=float(1.0 / D)
                        )
                        rsq_ = smp.tile([P, 1], f32, tag="rsq", name=f"rsq{b}_{i}")
                        nc.vector.reciprocal(rsq_, rms_)
                        hn = hnat.tile([P, D], f16, tag="h", name=f"h{b}_{i}")
                        nc.vector.tensor_scalar_mul(hn, xt_, rsq_)
                        nc.sync.dma_start(
                            h_d[b * 4 + i // 4][(i % 4) * P : (i % 4 + 1) * P, :], hn
                        )
                    hTt = hTp.tile([P, KC, TB], f16, tag="hT", name=f"hT{b}_{qc}")
                    for d in range(KC):
                        nc.sync.dma_start_transpose(
                            hTt[:, d, :], h_d[b * 4 + qc][:, d * P : (d + 1) * P]
                        )
                    qslc = slice(qc * TB, (qc + 1) * TB)
                    for h in range(HPC):
                        # Q projection + rope (even dims 0:64 = x1, odd = x2)
                        qp = qkps.tile([P, TB], f32, tag="qk", name=f"qp{b}_{qc}_{h}")
                        for d in range(KC):
                            nc.tensor.matmul(
                                qp,
                                wq_sb[:, d, h * P : (h + 1) * P],
                                hTt[:, d, :],
                                start=(d == 0),
                                stop=(d == KC - 1),
                            )
                        cq = cosq[:, qslc]
                        sq = sinq[:, qslc]
                        t1 = rtmp.tile([64, TB], f32, tag="t1", name=f"t1_{b}{qc}{h}")
                        t2 = rtmp.tile([64, TB], f32, tag="t2", name=f"t2_{b}{qc}{h}")
                        t3 = rtmp.tile([64, TB], f32, tag="t3", name=f"t3_{b}{qc}{h}")
                        t4 = rtmp.tile([64, TB], f32, tag="t4", name=f"t4_{b}{qc}{h}")
                        nc.vector.tensor_mul(t1, qp[0:64, :], cq)
                        nc.vector.tensor_mul(t2, qp[64:P, :], sq)
                        nc.vector.tensor_tensor(
                            QR[h][0:64, qslc], t1, t2, OP.subtract
                        )
                        nc.vector.tensor_mul(t3, qp[0:64, :], sq)
                        nc.vector.tensor_mul(t4, qp[64:P, :], cq)
                        nc.vector.tensor_tensor(QR[h][64:P, qslc], t3, t4, OP.add)
                        # K projection (no rope)
                        kp = qkps.tile([P, TB], f32, tag="qk", name=f"kp{b}_{qc}_{h}")
                        for d in range(KC):
                            nc.tensor.matmul(
                                kp,
                                wk_sb[:, d, h * P : (h + 1) * P],
                                hTt[:, d, :],
                                start=(d == 0),
                                stop=(d == KC - 1),
                            )
                        nc.vector.tensor_copy(KK[h][:, qslc], kp)
                    # V projection + rope, natural layout [tok, head, dh]
                    for tt in range(4):
                        gt_ = qc * 4 + tt
                        vp_ = vps.tile(
                            [P, HPC, P], f32, tag="v", name=f"vp{b}_{qc}_{tt}"
                        )
                        for d in range(KC):
                            nc.tensor.matmul(
                                vp_.rearrange("p h k -> p (h k)"),
                                hTt[:, d, tt * P : (tt + 1) * P],
                                wv_sb[:, d, :],
                                start=(d == 0),
                                stop=(d == KC - 1),
                            )
                        cvb = cosv[:, gt_, None, :].to_broadcast([P, HPC, 64])
                        svb = sinv[:, gt_, None, :].to_broadcast([P, HPC, 64])
                        v1 = vtmp.tile([P, HPC, 64], f32, tag="v1", name=f"v1_{b}{gt_}")
                        v2 = vtmp.tile([P, HPC, 64], f32, tag="v2", name=f"v2_{b}{gt_}")
                        v3 = vtmp.tile([P, HPC, 64], f32, tag="v3", name=f"v3_{b}{gt_}")
                        v4 = vtmp.tile([P, HPC, 64], f32, tag="v4", name=f"v4_{b}{gt_}")
                        nc.vector.tensor_mul(v1, vp_[:, :, 0:64], cvb)
                        nc.vector.tensor_mul(v2, vp_[:, :, 64:P], svb)
                        nc.vector.tensor_tensor(
                            VV[:, gt_, :, 0:64], v1, v2, OP.subtract
                        )
                        nc.vector.tensor_mul(v3, vp_[:, :, 0:64], svb)
                        nc.vector.tensor_mul(v4, vp_[:, :, 64:P], cvb)
                        nc.vector.tensor_tensor(VV[:, gt_, :, 64:P], v3, v4, OP.add)

                # causal attention, transposed orientation: AVT[dh, q]
                stage_t = {}
                for qc in range(4):
                    j = b * 4 + qc
                    for h in range(HPC):
                        stage_t[(qc, h)] = stg.tile(
                            [P, TB], f16, tag=f"stage{h}", name=f"stage{j}_{h}"
                        )
                for h in range(HPC):
                    for qc in range(4):
                        qslc = slice(qc * TB, (qc + 1) * TB)
                        avp_ = avps.tile([P, TB], f32, tag="av", name=f"av{b}{h}{qc}")
                        dnp_ = dnps.tile([1, TB], f32, tag="dn", name=f"dn{b}{h}{qc}")
                        nkc = 4 * qc + 4
                        for kc in range(nkc):
                            scp_ = scps.tile(
                                [P, TB], f32, tag="sc", name=f"sc{b}{h}{qc}_{kc}"
                            )
                            nc.tensor.matmul(
                                scp_,
                                KK[h][:, kc * P : (kc + 1) * P],
                                QR[h][:, qslc],
                                start=True,
                                stop=True,
                            )
                            ex = exps.tile(
                                [P, TB], f16, tag="ex", name=f"ex{b}{h}{qc}_{kc}"
                            )
                            nc.scalar.activation(ex, scp_, AF.Exp, scale=inv_sqrt_dh)
                            if kc >= 4 * qc:
                                nc.vector.tensor_mul(
                                    ex, ex, maskt[:, kc - 4 * qc, :]
                                )
                            nc.tensor.matmul(
                                avp_,
                                VV[:, kc, h, :],
                                ex,
                                start=(kc == 0),
                                stop=(kc == nkc - 1),
                            )
                            nc.tensor.matmul(
                                dnp_,
                                ones_c,
                                ex,
                                start=(kc == 0),
                                stop=(kc == nkc - 1),
                            )
                        rd_ = rdp.tile([1, TB], f32, tag="rd", name=f"rd{b}{h}{qc}")
                        nc.vector.reciprocal(rd_, dnp_)
                        rdB_ = rdBp.tile([P, TB], f32, tag="rdB", name=f"rB{b}{h}{qc}")
                        nc.gpsimd.partition_broadcast(rdB_, rd_)
                        nc.vector.tensor_mul(stage_t[(qc, h)], avp_, rdB_)
                        nc.sync.dma_start(a2a_in[h][b * 4 + qc], stage_t[(qc, h)])
                    if b == B - 1:
                        nc.gpsimd.collective_compute(
                            "AllToAll",
                            mybir.AluOpType.bypass,
                            replica_groups=[list(range(NC))],
                            ins=[a2a_in[h].opt()],
                            outs=[a2a_out[h].opt()],
                        )

        # ---------- phase 3: W_O + residual + rmsnorm2 + transpose ----------
        h2Tt = h2Tp.tile([P, KC, TB], f16)
        with ExitStack() as p3:
            x2p = p3.enter_context(tc.tile_pool(name="x2p", bufs=1))
            hoTp = p3.enter_context(tc.tile_pool(name="hoT", bufs=1))
            woep = p3.enter_context(tc.tile_pool(name="woe", bufs=2))
            xresp = p3.enter_context(tc.tile_pool(name="xres", bufs=1))
            h2p = p3.enter_context(tc.tile_pool(name="h2p", bufs=1))
            scr2 = p3.enter_context(tc.tile_pool(name="scr2", bufs=2))
            sm2 = p3.enter_context(tc.tile_pool(name="sm2", bufs=6))
            wops = p3.enter_context(tc.tile_pool(name="wops", bufs=3, space="PSUM"))
            trps = p3.enter_context(tc.tile_pool(name="trps", bufs=2, space="PSUM"))
            x2t = x2p.tile([P, 4, D], f32)
            b2_sb = x2p.tile([1, D], f32)
            nc.sync.dma_start(b2_sb, b2.ap())
            b2B = x2p.tile([P, D], f32)
            nc.gpsimd.partition_broadcast(b2B, b2_sb)
            hoTt = hoTp.tile([P, KC, TB], f16)
            for d in range(KC):
                nc.sync.dma_start(hoTt[:, d, :], a2a_out[d % 2][d // 2])
            xr = xresp.tile([P, 4, D], f32)
            nc.sync.dma_start(xr, x_res.rearrange("(i p) e -> p i e", p=P))
            for e in range(4):
                woe_t = woep.tile([P, KC, TB], f16, tag="woe", name=f"woe{e}")
                nc.sync.dma_start(
                    woe_t,
                    wo.rearrange("(c p) e -> p c e", p=P)[
                        :, :, e * TB : (e + 1) * TB
                    ],
                )
                for tt in range(4):
                    wp = wops.tile([P, TB], f32, tag="wo", name=f"wo{e}_{tt}")
                    for d in range(KC):
                        nc.tensor.matmul(
                            wp,
                            hoTt[:, d, tt * P : (tt + 1) * P],
                            woe_t[:, d, :],
                            start=(d == 0),
                            stop=(d == KC - 1),
                        )
                    nc.vector.tensor_tensor(
                        x2t[:, tt, e * TB : (e + 1) * TB],
                        wp,
                        xr[:, tt, e * TB : (e + 1) * TB],
                        OP.add,
                    )
            h2t = h2p.tile([P, 4, D], f16)
            for tt in range(4):
                s2 = scr2.tile([P, D], f32, tag="s2", name=f"s2_{tt}")
                ssq2 = sm2.tile([P, 1], f32, tag="ssq2", name=f"ssq2_{tt}")
                nc.scalar.activation(s2, x2t[:, tt, :], AF.Square, accum_out=ssq2)
                rms2 = sm2.tile([P, 1], f32, tag="rms2", name=f"rms2_{tt}")
                nc.scalar.activation(
                    rms2, ssq2, AF.Sqrt, bias=eps_t, scale=float(1.0 / D)
                )
                rsq2 = sm2.tile([P, 1], f32, tag="rsq2", name=f"rsq2_{tt}")
                nc.vector.reciprocal(rsq2, rms2)
                nc.vector.tensor_scalar_mul(h2t[:, tt, :], x2t[:, tt, :], rsq2)
            # fold B2 into x2 AFTER h2 is derived (out = x2 + B2 + mlp)
            for tt in range(4):
                nc.vector.tensor_tensor(x2t[:, tt, :], x2t[:, tt, :], b2B, OP.add)
            nc.sync.dma_start(x2_d.rearrange("(i p) e -> p i e", p=P), x2t)
            for d in range(KC):
                tp = trps.tile([P, TB], f16, tag="tp", name=f"tp{d}")
                for tt in range(4):
                    nc.tensor.transpose(
                        tp[:, tt * P : (tt + 1) * P],
                        h2t[:, tt, d * P : (d + 1) * P],
                        ident16,
                    )
                nc.vector.tensor_copy(h2Tt[:, d, :], tp)

        # ---------- phase 4: MLP ----------
        with ExitStack() as p4:
            w1p = p4.enter_context(tc.tile_pool(name="w1p", bufs=24))
            gtp = p4.enter_context(tc.tile_pool(name="gtp", bufs=1))
            w2p = p4.enter_context(tc.tile_pool(name="w2p", bufs=6))
            outp = p4.enter_context(tc.tile_pool(name="outp", bufs=1))
            x2lp = p4.enter_context(tc.tile_pool(name="x2l", bufs=3))
            w1v = w1.rearrange("(c p) f -> p c f", p=P)
            w2v = w2.rearrange("(c p) e -> p c e", p=P)
            gtt = gtp.tile([P, FC, TB], f16)
            m1ctx = ExitStack()
            m1ps = m1ctx.enter_context(tc.tile_pool(name="m1ps", bufs=3, space="PSUM"))
            for fg in range(16):
                tiles_fg = []
                for d in range(KC):
                    t = w1p.tile([P, TB], f16, tag="w1", name=f"w1_{fg}_{d}")
                    nc.sync.dma_start(t, w1v[:, d, fg * TB : (fg + 1) * TB])
                    tiles_fg.append(t)
                for fs in range(4):
                    f = fg * 4 + fs
                    mp = m1ps.tile([P, TB], f32, tag="m1", name=f"m1_{f}")
                    for d in range(KC):
                        nc.tensor.matmul(
                            mp,
                            tiles_fg[d][:, fs * P : (fs + 1) * P],
                            h2Tt[:, d, :],
                            start=(d == 0),
                            stop=(d == KC - 1),
                        )
                    nc.scalar.activation(
                        gtt[:, f, :], mp, AF.Relu, bias=b1_sb[:, f : f + 1]
                    )
            m1ctx.close()
            m2ctx = ExitStack()
            m2ps = m2ctx.enter_context(tc.tile_pool(name="m2ps", bufs=2, space="PSUM"))
            outt = outp.tile([P, 4, D], f32)
            for e in range(4):
                mps = []
                for tt in range(4):
                    m_ = m2ps.tile([P, TB], f32, tag=f"m2_{tt}", name=f"m2_{e}_{tt}")
                    mps.append(m_)
                for f in range(FC):
                    w2t = w2p.tile([P, TB], f16, tag="w2", name=f"w2_{e}_{f}")
                    nc.sync.dma_start(w2t, w2v[:, f, e * TB : (e + 1) * TB])
                    for tt in range(4):
                        nc.tensor.matmul(
                            mps[tt],
                            gtt[:, f, tt * P : (tt + 1) * P],
                            w2t,
                            start=(f == 0),
                            stop=(f == FC - 1),
                        )
                for tt in range(4):
                    x2l = x2lp.tile([P, TB], f32, tag="x2l", name=f"x2l_{e}_{tt}")
                    nc.sync.dma_start(
                        x2l,
                        x2_d.rearrange("(i p) e -> p i e", p=P)[
                            :, tt, e * TB : (e + 1) * TB
                        ],
                    )
                    nc.vector.tensor_tensor(
                        outt[:, tt, e * TB : (e + 1) * TB],
                        mps[tt],
                        x2l,
                        OP.add,
                    )
                nc.sync.dma_start(
                    out_d.rearrange("(i p) e -> p i e", p=P)[
                        :, :, e * TB : (e + 1) * TB
                    ],
                    outt[:, :, e * TB : (e + 1) * TB],
                )
            m2ctx.close()

    nc.compile()
    return nc


def _host_inputs(inputs):
    x = np.asarray(inputs["x"], np.float32)
    Wq = np.asarray(inputs["Wq"], np.float32)
    Wk = np.asarray(inputs["Wk"], np.float32)
    Wv = np.asarray(inputs["Wv"], np.float32)
    W_O = np.asarray(inputs["W_O"], np.float32)
    scale1 = np.asarray(inputs["scale1"], np.float32)
    scale2 = np.asarray(inputs["scale2"], np.float32)
    W1 = np.asarray(inputs["W1"], np.float32)
    B1 = np.asarray(inputs["B1"], np.float32)
    W2 = np.asarray(inputs["W2"], np.float32)
    B2 = np.asarray(inputs["B2"], np.float32)

    perm = np.concatenate([np.arange(0, DH, 2), np.arange(1, DH, 2)])
    # fold rmsnorm scales into the following matmuls
    Wq_s = Wq * scale1[None, :, None]
    Wk_s = Wk * scale1[None, :, None]
    Wv_s = Wv * scale1[None, :, None]
    W1_s = W1 * scale2[:, None]

    # W_O rows reordered to match the permuted, head-major layout of HO.T
    row_order = np.concatenate([h * DH + perm for h in range(H)])
    wo_c = np.ascontiguousarray(W_O[row_order, :]).astype(np.float16)
    w1_c = W1_s.astype(np.float16)
    w2_c = W2.astype(np.float16)
    b1s_c = np.ascontiguousarray(B1.reshape(FC, P).T)
    b2_c = B2.reshape(1, D)

    # rope tables
    pos = np.arange(S, dtype=np.float64)
    pidx = np.arange(64, dtype=np.float64)
    theta_p = 1.0 / THETA ** (2.0 * pidx / DH)
    ang = pos[None, :] * theta_p[:, None]  # [64, S]
    cos_qt_c = np.cos(ang).astype(np.float16)
    sin_qt_c = np.sin(ang).astype(np.float16)
    cos_v_c = np.ascontiguousarray(cos_qt_c.T)
    sin_v_c = np.ascontiguousarray(sin_qt_c.T)

    ii = np.arange(P)[:, None]
    jj = np.arange(TB)[None, :]
    masks_c = np.stack(
        [(ii + P * m <= jj).astype(np.float16) for m in range(4)]
    )

    in_maps = []
    for c in range(NC):
        b, r = c // 4, c % 4
        heads = [HPC * c, HPC * c + 1]
        wq_c = np.concatenate([Wq_s[h][:, perm] for h in heads], 1).astype(np.float16)
        wk_c = np.concatenate([Wk_s[h][:, perm] for h in heads], 1).astype(np.float16)
        wv_c = np.concatenate([Wv_s[h][:, perm] for h in heads], 1).astype(np.float16)
        in_maps.append(
            {
                "x_nat": x,
                "x_res": np.ascontiguousarray(x[b, r * TB : (r + 1) * TB, :]),
                "wq": np.ascontiguousarray(wq_c),
                "wk": np.ascontiguousarray(wk_c),
                "wv": np.ascontiguousarray(wv_c),
                "wo": wo_c,
                "w1": w1_c,
                "w2": w2_c,
                "b1s": b1s_c,
                "b2": b2_c,
                "cos_qt": cos_qt_c,
                "sin_qt": sin_qt_c,
                "cos_v": cos_v_c,
                "sin_v": sin_v_c,
                "masks": masks_c,
            }
        )
    return in_maps


def kernel(**inputs):
    from concourse.bass_utils import run_bass_kernel_spmd

    trace = bool(os.environ.get("BASS_KERNEL_TRACE"))
    if trace:
        _install_ntff_hook()

    if "nc" not in _CACHE:
        _CACHE["nc"] = _build()
    nc = _CACHE["nc"]

    in_maps = _host_inputs(inputs)
    r = run_bass_kernel_spmd(nc, in_maps, list(range(NC)), trace=trace)
    kernel.last_exec_time_ns = r.exec_time_ns

    out = np.empty((B, S, D), np.float32)
    for c in range(NC):
        b, rr = c // 4, c % 4
        out[b, rr * TB : (rr + 1) * TB, :] = r.results[c]["out"]
    return out


kernel.last_exec_time_ns = None

